# revision 1
# baseline (speedup 1.0000x reference)
"""GCN encoder (2x GCNConv+BN+ReLU, global mean pool) on 8 TRN2 NeuronCores.

Self-contained Bass/Tile kernel. Sharding: nodes (and incident edges, keyed
by dst) are partitioned in contiguous ranges across the 8 cores; weights are
replicated. Each layer's dense transform runs on the owning core's shard and
the full transformed table is assembled with an AllGather so every core can
gather arbitrary source rows for its local edges with dma_gather. The
scatter-add aggregation is a sequence of one-hot matmuls accumulating in
PSUM, one PSUM tile per 128-dst-node block. Per-graph pooled sums are
AllReduced and scaled by 1/count on device.

Host-side work: index/schedule planning (edge bucketing by (core, dst
block), int16 index packing for dma_gather, degree/norm computation, BN
folding into the weights) and per-core input shard prep.
"""

import math
from contextlib import ExitStack
from dataclasses import dataclass

import numpy as np

import concourse.bass as bass  # noqa: F401
import concourse.mybir as mybir
import concourse.tile as tile
from concourse import bacc, bass_utils
from concourse.masks import make_identity

P = 128
F16 = mybir.dt.float16
F32 = mybir.dt.float32
I16 = mybir.dt.int16

N_CORES = 8
LO_SPLIT = 32768
N_GRAPHS = 128
BN_EPS = 1e-5


@dataclass
class Plan:
    n_cores: int
    N: int
    H: int
    D: int
    G: int
    npc: int
    B: int
    lo_split: int
    TA: np.ndarray
    TB: np.ndarray
    T: np.ndarray
    off: np.ndarray
    Ttot: int
    idx16: list
    drel: list
    disv: list
    dinv: list
    xt: list
    bt: list
    w1p: np.ndarray
    w2p: np.ndarray
    sh1: np.ndarray
    sh2: np.ndarray
    cinv: np.ndarray


def plan_gcn(x, edge_index, batch, W1, b1, gamma1, beta1, mean1, var1,
             W2, b2, gamma2, beta2, mean2, var2,
             n_cores=N_CORES, lo_split=LO_SPLIT, bn_eps=BN_EPS,
             n_graphs=N_GRAPHS):
    N_real, D = x.shape
    H = W1.shape[1]
    npc = math.ceil(N_real / n_cores)
    N = npc * n_cores
    B = math.ceil(npc / P)
    G = n_graphs

    src = edge_index[0].astype(np.int64)
    dst = edge_index[1].astype(np.int64)
    deg = np.bincount(dst, minlength=N_real).astype(np.float64) + 1.0
    dis = 1.0 / np.sqrt(deg)
    loop = np.arange(N_real, dtype=np.int64)
    src_all = np.concatenate([src, loop])
    dst_all = np.concatenate([dst, loop])
    norm_all = (dis[src_all] * dis[dst_all]).astype(np.float32)

    core = dst_all // npc
    ldst = dst_all - core * npc
    blk = ldst // P
    drel = ldst - blk * P
    seg = (src_all >= lo_split).astype(np.int64)

    key = (core * B + blk) * 2 + seg
    ngroups = n_cores * B * 2
    counts = np.bincount(key, minlength=ngroups)
    tiles = -(-counts.reshape(n_cores, B, 2) // P)
    TA = tiles[:, :, 0].max(axis=0)
    TB = tiles[:, :, 1].max(axis=0)
    T = TA + TB
    off = np.concatenate([[0], np.cumsum(T)]).astype(np.int64)
    Ttot = int(off[-1])

    # sort by (core, block, segment) and by src within each group: ascending
    # gather addresses give the HBM controller row-hit/bank-parallel friendly
    # request streams, and duplicate sources become row-buffer hits.
    order = np.lexsort((src_all, key))
    s_src = src_all[order]
    s_norm = norm_all[order]
    s_drel = drel[order]
    s_core = core[order]
    s_blk = blk[order]
    s_seg = seg[order]
    s_key = key[order]
    group_starts = np.concatenate([[0], np.cumsum(counts)])[:-1]
    rank = np.arange(len(order)) - group_starts[s_key]
    slot_base = off[s_blk] * P + np.where(s_seg == 1, TA[s_blk] * P, 0)
    slot = slot_base + rank
    idx_val = np.where(s_seg == 1, s_src - lo_split, s_src).astype(np.int32)

    idx_slots = np.zeros((n_cores, Ttot * P), dtype=np.int32)
    drel_slots = np.full((n_cores, Ttot * P), -1.0, dtype=np.float32)
    idx_slots[s_core, slot] = idx_val
    drel_slots[s_core, slot] = s_drel.astype(np.float32)

    s1 = (gamma1 / np.sqrt(var1 + bn_eps)).astype(np.float64)
    s2 = (gamma2 / np.sqrt(var2 + bn_eps)).astype(np.float64)
    w1p = (W1.astype(np.float64) * s1[None, :]).astype(np.float16)
    w2p_sq = (W2.astype(np.float64) * s2[None, :]).astype(np.float16)
    w2p = np.concatenate([w2p_sq[:P, :], w2p_sq[P:2 * P, :]], axis=1)
    sh1 = (((b1 - mean1) * s1) + beta1).astype(np.float16)[None, :]
    sh2 = (((b2 - mean2) * s2) + beta2).astype(np.float16)[None, :]

    cnt = np.bincount(batch.astype(np.int64), minlength=128).astype(np.float64)
    cinv = (1.0 / np.maximum(cnt, 1.0)).astype(np.float32)[:, None]

    idx16, drel_l, disv_l, dinv_l, xt, bt = [], [], [], [], [], []
    x_pad = np.zeros((N, D), dtype=np.float16)
    x_pad[:N_real] = x.astype(np.float16)
    batch_pad = np.full(N, -1.0, dtype=np.float32)
    batch_pad[:N_real] = batch.astype(np.float32)
    dis_pad = np.zeros(N, dtype=np.float64)
    dis_pad[:N_real] = dis
    for c in range(n_cores):
        a = idx_slots[c].reshape(Ttot * 8, 16).T.astype(np.int16)
        idx16.append(np.tile(a, (8, 1)))
        drel_l.append(np.ascontiguousarray(
            drel_slots[c].reshape(Ttot, P).T.astype(np.float16)))
        dloc = np.zeros(B * P, dtype=np.float64)
        dloc[:npc] = dis_pad[c * npc:(c + 1) * npc]
        disv_l.append(np.ascontiguousarray(
            dloc.reshape(B, P).T.astype(np.float32)))
        dinv = np.where(dloc > 0, 1.0 / np.maximum(dloc, 1e-30), 0.0)
        dinv_l.append(dinv.astype(np.float16)[None, :])
        xt.append(np.ascontiguousarray(x_pad[c * npc:(c + 1) * npc].T))
        btc = np.full(B * P, -1.0, dtype=np.float32)
        btc[:npc] = batch_pad[c * npc:(c + 1) * npc]
        bt.append(np.ascontiguousarray(
            btc.reshape(B, P).T.astype(np.float16)))

    return Plan(n_cores=n_cores, N=N, H=H, D=D, G=G, npc=npc, B=B,
                lo_split=lo_split, TA=TA, TB=TB, T=T, off=off, Ttot=Ttot,
                idx16=idx16, drel=drel_l, disv=disv_l, dinv=dinv_l,
                xt=xt, bt=bt,
                w1p=w1p, w2p=w2p, sh1=sh1, sh2=sh2, cinv=cinv)


def in_maps_for(plan: Plan):
    return [{
        "xt": plan.xt[c],
        "idx": plan.idx16[c],
        "drel": plan.drel[c],
        "disv": plan.disv[c],
        "dinv": plan.dinv[c],
        "bt": plan.bt[c],
        "cinv": plan.cinv,
        "w1": plan.w1p,
        "w2": plan.w2p,
        "sh1": plan.sh1,
        "sh2": plan.sh2,
    } for c in range(plan.n_cores)]


def declare_inputs(nc, plan: Plan):
    t = {}
    t["xt"] = nc.dram_tensor("xt", [P, plan.npc], F16, kind="ExternalInput").ap()
    t["idx"] = nc.dram_tensor("idx", [P, 8 * plan.Ttot], I16,
                              kind="ExternalInput").ap()
    t["drel"] = nc.dram_tensor("drel", [P, plan.Ttot], F16,
                               kind="ExternalInput").ap()
    t["disv"] = nc.dram_tensor("disv", [P, plan.B], F32,
                               kind="ExternalInput").ap()
    t["dinv"] = nc.dram_tensor("dinv", [1, plan.B * P], F16,
                               kind="ExternalInput").ap()
    t["bt"] = nc.dram_tensor("bt", [P, plan.B], F16, kind="ExternalInput").ap()
    t["cinv"] = nc.dram_tensor("cinv", [P, 1], F32, kind="ExternalInput").ap()
    t["w1"] = nc.dram_tensor("w1", [plan.D, plan.H], F16,
                             kind="ExternalInput").ap()
    t["w2"] = nc.dram_tensor("w2", [P, 2 * plan.H], F16,
                             kind="ExternalInput").ap()
    t["sh1"] = nc.dram_tensor("sh1", [1, plan.H], F16, kind="ExternalInput").ap()
    t["sh2"] = nc.dram_tensor("sh2", [1, plan.H], F16, kind="ExternalInput").ap()
    t["out"] = nc.dram_tensor("out", [P, plan.H], F32, kind="ExternalOutput").ap()
    return t


def build_gcn(tc: tile.TileContext, io: dict, plan: Plan, repeat: int = 1,
              fake_cc: bool = False):
    nc = tc.nc
    H = plan.H
    B = plan.B
    npc = plan.npc
    rg = [list(range(plan.n_cores))]
    Tmax = int(plan.T.max())

    aspace = "Shared" if plan.n_cores > 4 else "Local"
    cc0_in = nc.dram_tensor("cc0_in", [npc, H], F16, kind="Internal").ap()
    h1_full = nc.dram_tensor("h1_full", [plan.N, H], F16, kind="Internal",
                             addr_space=aspace).ap()
    cc1_in = nc.dram_tensor("cc1_in", [npc, H], F16, kind="Internal").ap()
    h2_full = nc.dram_tensor("h2_full", [plan.N, H], F16, kind="Internal",
                             addr_space=aspace).ap()
    cc2_in = nc.dram_tensor("cc2_in", [P, H], F32, kind="Internal").ap()
    pool_out = nc.dram_tensor("pool_out", [P, H], F32, kind="Internal",
                              addr_space=aspace).ap()

    with ExitStack() as ctx:
        const = ctx.enter_context(tc.tile_pool(name="const", bufs=1))
        resident = ctx.enter_context(tc.tile_pool(name="resident", bufs=1))
        gpool = ctx.enter_context(tc.tile_pool(name="gath", bufs=3))
        spool = ctx.enter_context(tc.tile_pool(name="smat", bufs=3))
        work = ctx.enter_context(tc.tile_pool(name="work", bufs=3))
        zpool = ctx.enter_context(tc.tile_pool(name="zt", bufs=3))
        psum_a = ctx.enter_context(tc.tile_pool(name="psA", bufs=2, space="PSUM"))
        psum_t = ctx.enter_context(tc.tile_pool(name="psT", bufs=2, space="PSUM"))
        psum_h = ctx.enter_context(tc.tile_pool(name="psH", bufs=2, space="PSUM"))
        psum_p = ctx.enter_context(tc.tile_pool(name="psP", bufs=1, space="PSUM"))

        # ---- constants ----
        w1_sb = const.tile([plan.D, H], F16)
        nc.sync.dma_start(out=w1_sb[:], in_=io["w1"][:])
        w2_sb = const.tile([P, 2 * H], F16)
        nc.sync.dma_start(out=w2_sb[:], in_=io["w2"][:])
        sh1_sb = const.tile([1, H], F16)
        nc.sync.dma_start(out=sh1_sb[:], in_=io["sh1"][:])
        sh2_sb = const.tile([1, H], F16)
        nc.sync.dma_start(out=sh2_sb[:], in_=io["sh2"][:])
        cinv_sb = const.tile([P, 1], F32)
        nc.sync.dma_start(out=cinv_sb[:], in_=io["cinv"][:])
        bt_sb = const.tile([P, B], F16)
        nc.sync.dma_start(out=bt_sb[:], in_=io["bt"][:])
        disv_sb = const.tile([P, B], F32)
        nc.sync.dma_start(out=disv_sb[:], in_=io["disv"][:])
        dinv_sb = const.tile([1, B * P], F16)
        nc.sync.dma_start(out=dinv_sb[:], in_=io["dinv"][:])
        ident = const.tile([P, P], F16)
        make_identity(nc, ident[:])
        iota = const.tile([P, Tmax * P], F16)
        nc.gpsimd.iota(iota[:], pattern=[[0, Tmax], [1, P]], base=0,
                       channel_multiplier=0, allow_small_or_imprecise_dtypes=True)

        # ---- resident inputs ----
        xt_sb = resident.tile([P, npc], F16)
        nc.sync.dma_start(out=xt_sb[:], in_=io["xt"][:])
        idx_sb = resident.tile([P, 8 * plan.Ttot], I16)
        nc.sync.dma_start(out=idx_sb[:], in_=io["idx"][:])
        drel_sb = resident.tile([P, plan.Ttot], F16)
        nc.sync.dma_start(out=drel_sb[:], in_=io["drel"][:])

        def layer(h_table, sh_sb, is_last, pool_ps):
            for b in range(B):
                TA_b = int(plan.TA[b])
                TB_b = int(plan.TB[b])
                T_b = TA_b + TB_b
                ob = int(plan.off[b])
                blen = min(P, npc - b * P)
                gath = gpool.tile([P, T_b * H], F16, tag="gath")
                g3 = gath[:].rearrange("p (t h) -> p t h", h=H)
                if TA_b:
                    nc.gpsimd.dma_gather(
                        out_ap=g3[:, :TA_b, :], in_ap=h_table[:, :],
                        idxs_ap=idx_sb[:, 8 * ob: 8 * (ob + TA_b)],
                        num_idxs=TA_b * P, num_idxs_reg=TA_b * P, elem_size=H,
                        single_packet=False, queue_num=(2 * b) % 4)
                if TB_b:
                    nc.gpsimd.dma_gather(
                        out_ap=g3[:, TA_b:T_b, :],
                        in_ap=h_table[plan.lo_split:, :],
                        idxs_ap=idx_sb[:, 8 * (ob + TA_b): 8 * (ob + T_b)],
                        num_idxs=TB_b * P, num_idxs_reg=TB_b * P, elem_size=H,
                        single_packet=False, queue_num=(2 * b + 1) % 4)
                smat = spool.tile([P, T_b * P], F16, tag="smat")
                s3 = smat[:].rearrange("p (t j) -> p t j", j=P)
                drel_b = drel_sb[:, ob:ob + T_b].rearrange("p t -> p t ()")
                i3 = iota[:, :T_b * P].rearrange("p (t j) -> p t j", j=P)
                nc.vector.tensor_tensor(
                    out=s3, in0=i3, in1=drel_b.to_broadcast([P, T_b, P]),
                    op=mybir.AluOpType.is_equal)
                ps = psum_a.tile([P, H], F32, tag="agg")
                for t in range(T_b):
                    nc.tensor.matmul(out=ps[:], lhsT=smat[:, t * P:(t + 1) * P],
                                     rhs=g3[:, t, :], start=(t == 0), stop=False)
                nc.tensor.matmul(out=ps[:], lhsT=dinv_sb[:1, b * P:(b + 1) * P],
                                 rhs=sh_sb[:1, :], start=False, stop=True)
                zsb = work.tile([P, H], F16, tag="zsb")
                nc.scalar.activation(out=zsb[:], in_=ps[:],
                                     func=mybir.ActivationFunctionType.Relu,
                                     scale=disv_sb[:, b:b + 1])
                if not is_last:
                    ph = psum_h.tile([P, H], F32, tag="h2ps")
                    for hf in range(2):
                        pt = psum_t.tile([P, P], F16, tag="tps")
                        nc.tensor.transpose(out=pt[:],
                                            in_=zsb[:, hf * P:(hf + 1) * P],
                                            identity=ident[:])
                        zt = zpool.tile([P, P], F16, tag="ztile")
                        nc.vector.tensor_copy(out=zt[:], in_=pt[:])
                        nc.tensor.matmul(out=ph[:], lhsT=zt[:],
                                         rhs=w2_sb[:, hf * H:(hf + 1) * H],
                                         start=(hf == 0), stop=(hf == 1))
                    h2sb = work.tile([P, H], F16, tag="h2sb")
                    nc.scalar.activation(out=h2sb[:blen, :], in_=ph[:blen, :],
                                         func=mybir.ActivationFunctionType.Copy,
                                         scale=disv_sb[:blen, b:b + 1])
                    nc.sync.dma_start(out=cc1_in[b * P: b * P + blen, :],
                                      in_=h2sb[:blen, :])
                else:
                    bmat = zpool.tile([P, P], F16, tag="bmat")
                    nc.vector.tensor_tensor(
                        out=bmat[:], in0=iota[:, :P],
                        in1=bt_sb[:, b:b + 1].to_broadcast([P, P]),
                        op=mybir.AluOpType.is_equal)
                    nc.tensor.matmul(out=pool_ps[:], lhsT=bmat[:], rhs=zsb[:],
                                     start=(b == 0), stop=(b == B - 1))

        def run_pipeline():
            # transform: h1_local = x_shard @ W1'
            for m in range(B):
                moff = m * P
                mlen = min(P, npc - moff)
                ps = psum_a.tile([P, H], F32, tag="agg")
                nc.tensor.matmul(out=ps[:mlen, :],
                                 lhsT=xt_sb[:, moff:moff + mlen],
                                 rhs=w1_sb[:], start=True, stop=True)
                hsb = work.tile([P, H], F16, tag="h1sb")
                nc.scalar.activation(out=hsb[:mlen, :], in_=ps[:mlen, :],
                                     func=mybir.ActivationFunctionType.Copy,
                                     scale=disv_sb[:mlen, m:m + 1])
                nc.sync.dma_start(out=cc0_in[moff:moff + mlen, :],
                                  in_=hsb[:mlen, :])
            if fake_cc:
                for r in range(plan.n_cores):
                    nc.sync.dma_start(out=h1_full[r * npc:(r + 1) * npc, :],
                                      in_=cc0_in[:, :])
            else:
                nc.gpsimd.collective_compute(
                    "AllGather", mybir.AluOpType.bypass, replica_groups=rg,
                    ins=[cc0_in[:]], outs=[h1_full[:]])
            pool_ps = psum_p.tile([P, H], F32, tag="pps")
            layer(h1_full, sh1_sb, False, None)
            if fake_cc:
                for r in range(plan.n_cores):
                    nc.sync.dma_start(out=h2_full[r * npc:(r + 1) * npc, :],
                                      in_=cc1_in[:, :])
            else:
                nc.gpsimd.collective_compute(
                    "AllGather", mybir.AluOpType.bypass, replica_groups=rg,
                    ins=[cc1_in[:]], outs=[h2_full[:]])
            layer(h2_full, sh2_sb, True, pool_ps)
            pool_sb = work.tile([P, H], F32, tag="poolsb")
            nc.vector.tensor_copy(out=pool_sb[:], in_=pool_ps[:])
            nc.sync.dma_start(out=cc2_in[:], in_=pool_sb[:])
            if fake_cc:
                nc.sync.dma_start(out=pool_out[:, :], in_=cc2_in[:, :])
            else:
                nc.gpsimd.collective_compute(
                    "AllReduce", mybir.AluOpType.add, replica_groups=rg,
                    ins=[cc2_in[:]], outs=[pool_out[:]])
            res_sb = work.tile([P, H], F32, tag="ressb")
            nc.sync.dma_start(out=res_sb[:], in_=pool_out[:])
            osb = work.tile([P, H], F32, tag="osb")
            nc.vector.tensor_scalar_mul(out=osb[:], in0=res_sb[:],
                                        scalar1=cinv_sb[:, :1])
            nc.sync.dma_start(out=io["out"][:], in_=osb[:])

        for _rep in range(repeat):
            run_pipeline()


def make_nc(plan: Plan, debug=False, repeat=1, fake_cc=False):
    nc = bacc.Bacc("TRN2", target_bir_lowering=False, debug=debug,
                   num_devices=1 if fake_cc else plan.n_cores,
                   num_swdge_queues=4, dynamic_dma_scratch_size=32768)
    io = declare_inputs(nc, plan)
    with tile.TileContext(nc) as tc:
        build_gcn(tc, io, plan, repeat=repeat, fake_cc=fake_cc)
    nc.compile()
    return nc


_CACHE = {}


def _build(**inputs):
    inputs = {k: np.asarray(v) for k, v in inputs.items()}
    plan = plan_gcn(**inputs)
    key = (plan.N, plan.H, plan.D, tuple(plan.TA.tolist()),
           tuple(plan.TB.tolist()))
    if key not in _CACHE:
        _CACHE[key] = make_nc(plan)
    return _CACHE[key], plan


def kernel(**inputs) -> np.ndarray:
    nc, plan = _build(**inputs)
    in_maps = in_maps_for(plan)
    last_err = None
    for _attempt in range(2):
        try:
            res = bass_utils.run_bass_kernel_spmd(
                nc, in_maps, core_ids=list(range(plan.n_cores)))
            return res.results[0]["out"][:plan.G].astype(np.float32)
        except Exception as e:  # transient device/worker hiccup: retry once
            last_err = e
    raise last_err



# revision 2
# speedup vs baseline: 2.5135x; 2.5135x over previous
"""GCN encoder v2 (2x GCNConv+BN+ReLU, global mean pool) on 8 TRN2 NeuronCores.

Structure vs v1:
- Layer 1 aggregates FIRST from a replicated, dis-prescaled x table (fp16,
  256B rows) -> no first AllGather, half the gather bytes, and gathers start
  at T=0.
- h1 table (rows = ReLU(BN(aggx@W1))*dis) is stored/AllGathered in fp8e4m3,
  halving the collective and the layer-2 gather payload.
- One-hot scatter matrices are built per 128-slot tile with
  tensor_scalar(is_equal) against a per-partition drel scalar (DVE 4x mode)
  instead of one big broadcast tensor_tensor.
- Layer 2 aggregation matmuls use fp16 one-hot (lhsT) x fp8 gathered rows
  (rhs) directly.

Sharding: nodes (and incident edges, keyed by dst) partitioned in contiguous
ranges across 8 cores; weights replicated; per-graph pooled sums AllReduced.
"""

import math
from contextlib import ExitStack
from dataclasses import dataclass

import numpy as np

import concourse.bass as bass  # noqa: F401
import concourse.mybir as mybir
import concourse.tile as tile
from concourse import bacc, bass_utils
from concourse.masks import make_identity

P = 128
F16 = mybir.dt.float16
F32 = mybir.dt.float32
F8 = mybir.dt.float8e4
I16 = mybir.dt.int16

N_CORES = 8
LO_SPLIT = 32768
N_GRAPHS = 128
BN_EPS = 1e-5


@dataclass
class Plan:
    n_cores: int
    N: int
    H: int
    D: int
    G: int
    npc: int
    B: int
    lo_split: int
    TA: np.ndarray
    TB: np.ndarray
    T: np.ndarray
    off: np.ndarray
    Ttot: int
    cb: np.ndarray
    idx16: list
    drel: list
    disv: list
    bt: list
    cpn: list
    xloc: list
    cA: np.ndarray
    cB: np.ndarray
    xg: np.ndarray
    w1p: np.ndarray
    w2p: np.ndarray
    sh1: np.ndarray
    sh2: np.ndarray


def plan_gcn(x, edge_index, batch, W1, b1, gamma1, beta1, mean1, var1,
             W2, b2, gamma2, beta2, mean2, var2,
             n_cores=N_CORES, lo_split=LO_SPLIT, bn_eps=BN_EPS,
             n_graphs=N_GRAPHS, n_chunks=4):
    N_real, D = x.shape
    H = W1.shape[1]
    npc = math.ceil(N_real / n_cores)
    N = npc * n_cores
    B = math.ceil(npc / P)
    G = n_graphs

    src = edge_index[0].astype(np.int64)
    dst = edge_index[1].astype(np.int64)
    deg = np.bincount(dst, minlength=N_real).astype(np.float64) + 1.0
    dis = 1.0 / np.sqrt(deg)
    # self-loops are handled by a per-block contiguous HWDGE copy, not by
    # SWDGE gather descriptors; only real edges go through the gather path.
    src_all = src
    dst_all = dst

    # Chunk-major table positions: the h1/x tables are laid out
    # [chunk][core][rows] so each AllGather chunk lands contiguously and can
    # overlap layer-1 compute. cb = chunk row boundaries within a core shard.
    bpc = -(-B // n_chunks)
    cb = [min(k * bpc * P, npc) for k in range(n_chunks)] + [npc]
    cb = np.asarray(cb, dtype=np.int64)
    S = np.diff(cb)

    def pos_of(node):
        c = node // npc
        r = node - c * npc
        k = np.minimum(np.searchsorted(cb, r, side="right") - 1, n_chunks - 1)
        return cb[k] * n_cores + c * S[k] + (r - cb[k])

    src_pos = pos_of(src_all)

    core = dst_all // npc
    ldst = dst_all - core * npc
    blk = ldst // P
    drel = ldst - blk * P
    seg = (src_pos >= lo_split).astype(np.int64)

    key = (core * B + blk) * 2 + seg
    ngroups = n_cores * B * 2
    counts = np.bincount(key, minlength=ngroups)
    tiles = -(-counts.reshape(n_cores, B, 2) // P)
    TA = tiles[:, :, 0].max(axis=0)
    TB = tiles[:, :, 1].max(axis=0)
    cA = counts.reshape(n_cores, B, 2)[:, :, 0].max(axis=0)
    cB = counts.reshape(n_cores, B, 2)[:, :, 1].max(axis=0)
    T = TA + TB + 1  # +1: trailing self-loop tile per block
    off = np.concatenate([[0], np.cumsum(T)]).astype(np.int64)
    Ttot = int(off[-1])

    # sort by (core, block, segment) then src: ascending gather addresses.
    order = np.lexsort((src_pos, key))
    s_src = src_pos[order]
    s_drel = drel[order]
    s_core = core[order]
    s_blk = blk[order]
    s_seg = seg[order]
    s_key = key[order]
    group_starts = np.concatenate([[0], np.cumsum(counts)])[:-1]
    rank = np.arange(len(order)) - group_starts[s_key]
    slot_base = off[s_blk] * P + np.where(s_seg == 1, TA[s_blk] * P, 0)
    slot = slot_base + rank
    idx_val = np.where(s_seg == 1, s_src - lo_split, s_src).astype(np.int32)

    idx_slots = np.zeros((n_cores, Ttot * P), dtype=np.int32)
    drel_slots = np.full((n_cores, Ttot * P), -1.0, dtype=np.float32)
    idx_slots[s_core, slot] = idx_val
    drel_slots[s_core, slot] = s_drel.astype(np.float32)
    for b in range(B):
        sbase = (off[b] + TA[b] + TB[b]) * P
        drel_slots[:, sbase:sbase + P] = np.arange(P, dtype=np.float32)

    s1 = (gamma1 / np.sqrt(var1 + bn_eps)).astype(np.float64)
    s2 = (gamma2 / np.sqrt(var2 + bn_eps)).astype(np.float64)
    w1p = (W1.astype(np.float64) * s1[None, :]).astype(np.float16)
    w2p_sq = (W2.astype(np.float64) * s2[None, :]).astype(np.float16)
    w2p = np.concatenate([w2p_sq[:P, :], w2p_sq[P:2 * P, :]], axis=1)
    sh1 = (((b1 - mean1) * s1) + beta1).astype(np.float16)[None, :]
    sh2 = (((b2 - mean2) * s2) + beta2).astype(np.float16)[None, :]

    cnt = np.bincount(batch.astype(np.int64), minlength=n_graphs)
    cnt = cnt.astype(np.float64)
    cinv_g = 1.0 / np.maximum(cnt, 1.0)
    cinv_node = cinv_g[batch.astype(np.int64)]  # per-node 1/count

    # dis-prescaled x gather table in chunk-major position order (padded to
    # N rows), replicated per core.
    xg = np.zeros((N, D), dtype=np.float16)
    xg[pos_of(np.arange(N_real))] = (
        x.astype(np.float64) * dis[:, None]).astype(np.float16)

    dis_pad = np.zeros(N, dtype=np.float64)
    dis_pad[:N_real] = dis
    batch_pad = np.full(N, -1.0, dtype=np.float32)
    batch_pad[:N_real] = batch.astype(np.float32)

    xg_rowmajor = np.zeros((N, D), dtype=np.float16)
    xg_rowmajor[:N_real] = (x.astype(np.float64) * dis[:, None]).astype(np.float16)

    idx16, drel_l, disv_l, bt_l, cpn_l, xloc_l = [], [], [], [], [], []
    for c in range(n_cores):
        a = idx_slots[c].reshape(Ttot * 8, 16).T.astype(np.int16)
        idx16.append(np.tile(a, (8, 1)))
        drel_l.append(np.ascontiguousarray(
            drel_slots[c].reshape(Ttot, P).T.astype(np.float16)))
        dloc = np.zeros(B * P, dtype=np.float64)
        dloc[:npc] = dis_pad[c * npc:(c + 1) * npc]
        disv_l.append(np.ascontiguousarray(
            dloc.reshape(B, P).T.astype(np.float32)))
        btc = np.full(B * P, -1.0, dtype=np.float32)
        btc[:npc] = batch_pad[c * npc:(c + 1) * npc]
        bt_l.append(np.ascontiguousarray(
            btc.reshape(B, P).T.astype(np.float16)))
        cpn = np.zeros(B * P, dtype=np.float64)
        lo, hi = c * npc, min((c + 1) * npc, N_real)
        cpn[:hi - lo] = cinv_node[lo:hi]
        cpn_l.append(np.ascontiguousarray(
            cpn.reshape(B, P).T.astype(np.float32)))
        xl = np.zeros((B * P, D), dtype=np.float16)
        xl[:hi - lo] = xg_rowmajor[lo:hi]
        xloc_l.append(xl)

    return Plan(n_cores=n_cores, N=N, H=H, D=D, G=G, npc=npc, B=B,
                lo_split=lo_split, TA=TA, TB=TB, T=T, off=off, Ttot=Ttot,
                cb=cb, idx16=idx16, drel=drel_l, disv=disv_l, bt=bt_l,
                cpn=cpn_l, xg=xg, xloc=xloc_l, cA=cA, cB=cB,
                w1p=w1p, w2p=w2p, sh1=sh1, sh2=sh2)


def in_maps_for(plan: Plan):
    return [{
        "xg": plan.xg,
        "idx": plan.idx16[c],
        "drel": plan.drel[c],
        "disv": plan.disv[c],
        "bt": plan.bt[c],
        "cpn": plan.cpn[c],
        "xloc": plan.xloc[c],
        "w1": plan.w1p,
        "w2": plan.w2p,
        "sh1": plan.sh1,
        "sh2": plan.sh2,
    } for c in range(plan.n_cores)]


def declare_inputs(nc, plan: Plan):
    t = {}
    t["xg"] = nc.dram_tensor("xg", [plan.N, plan.D], F16,
                             kind="ExternalInput").ap()
    t["idx"] = nc.dram_tensor("idx", [P, 8 * plan.Ttot], I16,
                              kind="ExternalInput").ap()
    t["drel"] = nc.dram_tensor("drel", [P, plan.Ttot], F16,
                               kind="ExternalInput").ap()
    t["disv"] = nc.dram_tensor("disv", [P, plan.B], F32,
                               kind="ExternalInput").ap()
    t["bt"] = nc.dram_tensor("bt", [P, plan.B], F16, kind="ExternalInput").ap()
    t["cpn"] = nc.dram_tensor("cpn", [P, plan.B], F32,
                              kind="ExternalInput").ap()
    t["xloc"] = nc.dram_tensor("xloc", [plan.B * P, plan.D], F16,
                               kind="ExternalInput").ap()
    t["w1"] = nc.dram_tensor("w1", [plan.D, plan.H], F16,
                             kind="ExternalInput").ap()
    t["w2"] = nc.dram_tensor("w2", [P, 2 * plan.H], F16,
                             kind="ExternalInput").ap()
    t["sh1"] = nc.dram_tensor("sh1", [1, plan.H], F16, kind="ExternalInput").ap()
    t["sh2"] = nc.dram_tensor("sh2", [1, plan.H], F16, kind="ExternalInput").ap()
    t["out"] = nc.dram_tensor("out", [P, plan.H], F32, kind="ExternalOutput").ap()
    return t


def build_gcn(tc: tile.TileContext, io: dict, plan: Plan, repeat: int = 1,
              fake_cc: bool = False, gbufs: int = 4, sbufs: int = 3):
    GBUFS = gbufs
    nc = tc.nc
    H = plan.H
    B = plan.B
    npc = plan.npc
    rg = [list(range(plan.n_cores))]

    aspace = "Shared" if plan.n_cores > 4 else "Local"
    cb = plan.cb
    n_chunks = len(cb) - 1
    Sch = np.diff(cb)
    bpc = -(-B // n_chunks)
    Spad = [int(-(-int(s) // P) * P) for s in Sch]
    chunk_of = [int(np.searchsorted(cb, b * P, side="right") - 1)
                for b in range(B)]
    chunk_of = [min(k, n_chunks - 1) for k in chunk_of]
    last_blk = {}
    for b in range(B):
        last_blk[chunk_of[b]] = b
    cc1_k = [nc.dram_tensor(f"cc1_in{k}", [Spad[k], H], F8,
                            kind="Internal").ap()
             for k in range(n_chunks)]
    t2_full = nc.dram_tensor("t2_full", [plan.N, H], F8, kind="Internal",
                             addr_space=aspace).ap()
    cc2_in = nc.dram_tensor("cc2_in", [P, H], F32, kind="Internal").ap()
    pool_out = nc.dram_tensor("pool_out", [P, H], F32, kind="Internal",
                              addr_space=aspace).ap()

    with ExitStack() as ctx:
        const = ctx.enter_context(tc.tile_pool(name="const", bufs=1))
        resident = ctx.enter_context(tc.tile_pool(name="resident", bufs=1))
        gpool = ctx.enter_context(tc.tile_pool(name="gath", bufs=gbufs))
        spool = ctx.enter_context(tc.tile_pool(name="smat", bufs=sbufs))
        work = ctx.enter_context(tc.tile_pool(name="work", bufs=3))
        zpool = ctx.enter_context(tc.tile_pool(name="zt", bufs=3))
        psum_a = ctx.enter_context(tc.tile_pool(name="psA", bufs=2, space="PSUM"))
        psum_t = ctx.enter_context(tc.tile_pool(name="psT", bufs=2, space="PSUM"))
        psum_h = ctx.enter_context(tc.tile_pool(name="psH", bufs=2, space="PSUM"))
        psum_p = ctx.enter_context(tc.tile_pool(name="psP", bufs=1, space="PSUM"))

        # ---- constants ----
        w1_sb = const.tile([plan.D, H], F16)
        nc.sync.dma_start(out=w1_sb[:], in_=io["w1"][:])
        w2_sb = const.tile([P, 2 * H], F16)
        nc.sync.dma_start(out=w2_sb[:], in_=io["w2"][:])
        sh1_sb = const.tile([1, H], F16)
        nc.sync.dma_start(out=sh1_sb[:], in_=io["sh1"][:])
        sh2_sb = const.tile([1, H], F16)
        nc.sync.dma_start(out=sh2_sb[:], in_=io["sh2"][:])
        cpn_sb = const.tile([P, plan.B], F32)
        nc.sync.dma_start(out=cpn_sb[:], in_=io["cpn"][:])
        bt_sb = const.tile([P, plan.B], F16)
        nc.sync.dma_start(out=bt_sb[:], in_=io["bt"][:])
        disv_sb = const.tile([P, plan.B], F32)
        nc.sync.dma_start(out=disv_sb[:], in_=io["disv"][:])
        ident = const.tile([P, P], F16)
        make_identity(nc, ident[:])
        Tmax = int(plan.T.max())
        iota = const.tile([P, Tmax * P], F16)
        nc.gpsimd.iota(iota[:], pattern=[[0, Tmax], [1, P]], base=0,
                       channel_multiplier=0,
                       allow_small_or_imprecise_dtypes=True)
        ones1 = const.tile([1, P], F16)
        nc.vector.memset(ones1[:], 1.0)

        # ---- resident inputs (chunked so block 0's gathers start early) ----
        idx_sb = resident.tile([P, 8 * plan.Ttot], I16)
        drel_sb = resident.tile([P, plan.Ttot], F16)
        nld = 6
        lb = [plan.Ttot * i // nld for i in range(nld + 1)]
        for i in range(nld):
            nc.sync.dma_start(out=idx_sb[:, 8 * lb[i]:8 * lb[i + 1]],
                              in_=io["idx"][:, 8 * lb[i]:8 * lb[i + 1]])
            nc.sync.dma_start(out=drel_sb[:, lb[i]:lb[i + 1]],
                              in_=io["drel"][:, lb[i]:lb[i + 1]])

        qload = [0, 0, 0, 0]

        def pick_queue(ndesc):
            q = min(range(4), key=lambda i: qload[i])
            qload[q] += ndesc
            return q

        def gather_block(b, table, elem, dtype, tag, ragged):
            """Issue the (up to) two segment gathers for dst block b.

            ragged=True trims num_idxs to the real (cross-core max) count;
            trailing slots keep stale-but-finite data and are masked by the
            zero one-hot columns.
            """
            TA_b = int(plan.TA[b])
            TB_b = int(plan.TB[b])
            nA = int(plan.cA[b]) if ragged else TA_b * P
            nB = int(plan.cB[b]) if ragged else TB_b * P
            T_b = int(plan.T[b])  # includes the trailing self-loop tile
            ob = int(plan.off[b])
            gath = gpool.tile([P, T_b * elem], dtype, tag=tag)
            g3 = gath[:].rearrange("p (t h) -> p t h", h=elem)
            if TA_b:
                nc.gpsimd.dma_gather(
                    out_ap=g3[:, :TA_b, :], in_ap=table[:, :],
                    idxs_ap=idx_sb[:, 8 * ob: 8 * (ob + TA_b)],
                    num_idxs=nA, num_idxs_reg=nA, elem_size=elem,
                    single_packet=False, queue_num=pick_queue(nA))
            if TB_b:
                nc.gpsimd.dma_gather(
                    out_ap=g3[:, TA_b:TA_b + TB_b, :],
                    in_ap=table[plan.lo_split:, :],
                    idxs_ap=idx_sb[:, 8 * (ob + TA_b): 8 * (ob + T_b)],
                    num_idxs=nB, num_idxs_reg=nB, elem_size=elem,
                    single_packet=False, queue_num=pick_queue(nB))
            return g3, T_b, ob

        def build_smat(b, T_b, ob):
            smat = spool.tile([P, T_b * P], F16, tag="smat")
            s3 = smat[:].rearrange("p (t j) -> p t j", j=P)
            drel_b = drel_sb[:, ob:ob + T_b].rearrange("p t -> p t ()")
            i3 = iota[:, :T_b * P].rearrange("p (t j) -> p t j", j=P)
            nc.vector.tensor_tensor(
                out=s3, in0=i3, in1=drel_b.to_broadcast([P, T_b, P]),
                op=mybir.AluOpType.is_equal)
            return smat

        Tmax_g = int(plan.T.max())
        zpad = const.tile([P, H], F8)
        nc.vector.memset(zpad[:], 0.0)
        for k in range(n_chunks):
            pad = Spad[k] - int(Sch[k])
            if pad:
                nc.sync.dma_start(out=cc1_k[k][int(Sch[k]):Spad[k], :],
                                  in_=zpad[:pad, :])

        def run_pipeline():
            # ---------------- layer 1 ----------------
            for b in range(B):
                blen = min(P, npc - b * P)
                g3, T_b, ob = gather_block(b, io["xg"], plan.D, F16, "g",
                                           ragged=(b >= GBUFS))
                nc.scalar.dma_start(out=g3[:, T_b - 1, :],
                                    in_=io["xloc"][b * P:(b + 1) * P, :])
                smat = build_smat(b, T_b, ob)
                ps = psum_a.tile([P, plan.D], F32, tag="agg")
                for t in range(T_b):
                    nc.tensor.matmul(out=ps[:], lhsT=smat[:, t * P:(t + 1) * P],
                                     rhs=g3[:, t, :], start=(t == 0),
                                     stop=(t == T_b - 1))
                zsb = work.tile([P, plan.D], F16, tag="z1")
                nc.scalar.activation(out=zsb[:], in_=ps[:],
                                     func=mybir.ActivationFunctionType.Copy,
                                     scale=disv_sb[:, b:b + 1])
                pt = psum_t.tile([P, P], F16, tag="tps")
                nc.tensor.transpose(out=pt[:], in_=zsb[:], identity=ident[:])
                zt = zpool.tile([P, P], F16, tag="zt1")
                nc.vector.tensor_copy(out=zt[:], in_=pt[:])
                ph = psum_h.tile([P, H], F32, tag="hps")
                nc.tensor.matmul(out=ph[:], lhsT=zt[:], rhs=w1_sb[:],
                                 start=True, stop=False)
                nc.tensor.matmul(out=ph[:], lhsT=ones1[:1, :], rhs=sh1_sb[:1, :],
                                 start=False, stop=True)
                t2sb = work.tile([P, H], F8, tag="t2sb")
                nc.scalar.activation(out=t2sb[:blen, :], in_=ph[:blen, :],
                                     func=mybir.ActivationFunctionType.Relu,
                                     scale=disv_sb[:blen, b:b + 1])
                k = chunk_of[b]
                r0 = b * P - int(cb[k])
                nc.sync.dma_start(out=cc1_k[k][r0: r0 + blen, :],
                                  in_=t2sb[:blen, :])
                if b == last_blk[k]:
                    # last block of chunk k: AllGather it now, overlapping
                    # the remaining layer-1 blocks.
                    ob_k = int(cb[k]) * plan.n_cores
                    sz_k = int(Sch[k]) * plan.n_cores
                    if fake_cc:
                        for r in range(plan.n_cores):
                            nc.sync.dma_start(
                                out=t2_full[ob_k + r * int(Sch[k]):
                                            ob_k + (r + 1) * int(Sch[k]), :],
                                in_=cc1_k[k][:int(Sch[k]), :])
                    else:
                        nc.gpsimd.collective_compute(
                            "AllGather", mybir.AluOpType.bypass,
                            replica_groups=rg,
                            ins=[cc1_k[k][:int(Sch[k]), :]],
                            outs=[t2_full[ob_k: ob_k + sz_k]])

            # Re-zero gather buffers at the layer boundary: layer 2 views
            # them as fp8 and ragged gathers leave stale fp16 bytes that can
            # alias to fp8 NaN (0x7f/0xff); NaN*0 would poison the PSUM.
            # Overlaps the AllGather tail.
            for _i in range(GBUFS):
                g0 = gpool.tile([P, Tmax_g * plan.D], F16, tag="g")
                nc.vector.memset(g0[:], 0.0)

            # ---------------- layer 2 + pool ----------------
            pool_ps = psum_p.tile([P, H], F32, tag="pps")
            for b in range(B):
                g3, T_b, ob = gather_block(b, t2_full, H, F8, "g",
                                           ragged=True)
                k2 = chunk_of[b]
                r0 = b * P - int(cb[k2])
                nc.scalar.dma_start(out=g3[:, T_b - 1, :],
                                    in_=cc1_k[k2][r0: r0 + P, :])
                smat = build_smat(b, T_b, ob)
                ps2 = psum_a.tile([P, H], F32, tag="agg")
                for t in range(T_b):
                    nc.tensor.matmul(out=ps2[:], lhsT=smat[:, t * P:(t + 1) * P],
                                     rhs=g3[:, t, :], start=(t == 0),
                                     stop=(t == T_b - 1))
                zsb2 = work.tile([P, H], F16, tag="z2")
                nc.scalar.activation(out=zsb2[:], in_=ps2[:],
                                     func=mybir.ActivationFunctionType.Copy,
                                     scale=disv_sb[:, b:b + 1])
                ph2 = psum_h.tile([P, H], F32, tag="hps")
                for hf in range(2):
                    pt = psum_t.tile([P, P], F16, tag="tps")
                    nc.tensor.transpose(out=pt[:],
                                        in_=zsb2[:, hf * P:(hf + 1) * P],
                                        identity=ident[:])
                    zt = zpool.tile([P, P], F16, tag="zt2")
                    nc.vector.tensor_copy(out=zt[:], in_=pt[:])
                    nc.tensor.matmul(out=ph2[:], lhsT=zt[:],
                                     rhs=w2_sb[:, hf * H:(hf + 1) * H],
                                     start=(hf == 0), stop=False)
                nc.tensor.matmul(out=ph2[:], lhsT=ones1[:1, :], rhs=sh2_sb[:1, :],
                                 start=False, stop=True)
                h2sb = work.tile([P, H], F16, tag="h2sb")
                nc.scalar.activation(out=h2sb[:], in_=ph2[:],
                                     func=mybir.ActivationFunctionType.Relu,
                                     scale=cpn_sb[:, b:b + 1])
                bmat = zpool.tile([P, P], F16, tag="bmat")
                nc.vector.tensor_tensor(
                    out=bmat[:], in0=iota[:, :P],
                    in1=bt_sb[:, b:b + 1].to_broadcast([P, P]),
                    op=mybir.AluOpType.is_equal)
                nc.tensor.matmul(out=pool_ps[:], lhsT=bmat[:], rhs=h2sb[:],
                                 start=(b == 0), stop=(b == B - 1))

            pool_sb = work.tile([P, H], F32, tag="poolsb")
            nc.vector.tensor_copy(out=pool_sb[:], in_=pool_ps[:])
            nc.sync.dma_start(out=cc2_in[:], in_=pool_sb[:])
            if fake_cc:
                nc.sync.dma_start(out=pool_out[:, :], in_=cc2_in[:, :])
            else:
                nc.gpsimd.collective_compute(
                    "AllReduce", mybir.AluOpType.add, replica_groups=rg,
                    ins=[cc2_in[:]], outs=[pool_out[:]])
            nc.sync.dma_start(out=io["out"][:], in_=pool_out[:, :])

        for _rep in range(repeat):
            run_pipeline()


def make_nc(plan: Plan, debug=False, repeat=1, fake_cc=False, gbufs=4,
            sbufs=3):
    nc = bacc.Bacc("TRN2", target_bir_lowering=False, debug=debug,
                   num_devices=1 if fake_cc else plan.n_cores,
                   num_swdge_queues=4, dynamic_dma_scratch_size=32768)
    io = declare_inputs(nc, plan)
    with tile.TileContext(nc) as tc:
        build_gcn(tc, io, plan, repeat=repeat, fake_cc=fake_cc, gbufs=gbufs,
                  sbufs=sbufs)
    nc.compile()
    return nc


_CACHE = {}


def _build(**inputs):
    inputs = {k: np.asarray(v) for k, v in inputs.items()}
    plan = plan_gcn(**inputs)
    key = (plan.N, plan.H, plan.D, tuple(plan.TA.tolist()),
           tuple(plan.TB.tolist()))
    if key not in _CACHE:
        _CACHE[key] = make_nc(plan)
    return _CACHE[key], plan


def kernel(**inputs) -> np.ndarray:
    nc, plan = _build(**inputs)
    in_maps = in_maps_for(plan)
    last_err = None
    for _attempt in range(2):
        try:
            res = bass_utils.run_bass_kernel_spmd(
                nc, in_maps, core_ids=list(range(plan.n_cores)))
            return res.results[0]["out"][:plan.G].astype(np.float32)
        except Exception as e:  # transient device/worker hiccup: retry once
            last_err = e
    raise last_err
